# revision 1
# baseline (speedup 1.0000x reference)
"""CayleyLinear Trainium2 kernel.

Computes out = x @ Q + bias where Q = (I-A)^-1 (I+A) is the Cayley
transform of the skew-symmetric matrix built from `angles`.

Strategy (data-parallel over the batch dim, 8 NeuronCores):
  - Host: build A, solve for Q (1024x1024 — tiny vs the 68.7 GFLOP matmul),
    replicate Q/bias to every core, shard x along batch (8 -> 1 per core).
  - Host: pre-transpose each x shard to [K=1024, S=4096] so the contraction
    dim lands on SBUF partitions (avoids 256 on-device PE transposes/core).
  - Host: cast x/Q to fp16 (10-bit mantissa ~= the PE's f32r product
    precision; measured 2.8e-4 rel err vs 1.4e-4 for f32r) - halves DMA
    traffic and enables the fast 2-byte FWL weight-load path (216ns/MM,
    the exact warm floor, vs 227ns for 4-byte f32r weights).
  - Device: out[s, j] = sum_k xT[k, s] * Q[k, j] via 128x128x512 fp16
    matmuls (1 cycle/row), accumulating 8 k-tiles in fp32 PSUM, fused
    bias-add on the PSUM->SBUF copy (DVE), then DMA out as fp32.
"""

import numpy as np

DIM = 1024
B = 8
S = 4096
N_CORES = 8
P = 128
KT = DIM // P  # 8 k-tiles
S_SLAB = 512  # tokens DMA'd per slab
N_SLABS = S // S_SLAB  # 8
M_PER_SLAB = S_SLAB // P  # 4 matmul row-blocks per slab

_compiled_nc = None


def _build_kernel():
    import concourse.bass as bass
    import concourse.mybir as mybir
    import concourse.tile as tile
    from concourse import bacc

    f32 = mybir.dt.float32
    f32r = mybir.dt.float16

    nc = bacc.Bacc(
        "TRN2",
        target_bir_lowering=False,
        debug=False,
        num_devices=N_CORES,
        enable_partition_id=False,
    )

    xT_d = nc.dram_tensor("xT", [DIM, S], mybir.dt.float16, kind="ExternalInput").ap()
    q_d = nc.dram_tensor("q", [DIM, DIM], mybir.dt.float16, kind="ExternalInput").ap()
    bias_d = nc.dram_tensor("biasr", [P, DIM], f32, kind="ExternalInput").ap()
    out_d = nc.dram_tensor("out", [S, DIM], f32, kind="ExternalOutput").ap()

    xT_r = xT_d.rearrange("(kt p) s -> p kt s", p=P)  # [128, 8, 4096]
    q_r = q_d.rearrange("(kt p) j -> p kt j", p=P)  # [128, 8, 1024]

    with tile.TileContext(nc) as tc:
        with (
            tc.tile_pool(name="qpool", bufs=1) as qpool,
            tc.tile_pool(name="bpool", bufs=1) as bpool,
            tc.tile_pool(name="xpool", bufs=4) as xpool,
            tc.tile_pool(name="opool", bufs=4) as opool,
            tc.tile_pool(name="psum", bufs=1, space="PSUM") as psumpool,
        ):
            # Q tiles load per (k-tile, j-half) on the gpsimd queue, x
            # tiles on the sync queue (parallel issue). The first/last
            # slabs are half-size (256 tokens) to shrink the data wait
            # before the first matmul and the drain tail after the last.
            # Slab 0 runs kt-major so the first matmuls only need the
            # first-arriving tiles and Q's 4MB arrival overlaps a slab
            # of PE work; later slabs (x prefetched) run m-outer so DVE
            # drains and output stores spread evenly.
            q_tiles = [[None] * 2 for _ in range(KT)]

            def mm(ps, xts, kt, m, is_last_kt):
                lhs = xts[kt][:, m * P : (m + 1) * P]
                for jh in range(2):
                    nc.tensor.matmul(
                        ps[:, jh * 512 : (jh + 1) * 512],
                        lhs,
                        q_tiles[kt][jh][:],
                        start=(kt == 0),
                        stop=is_last_kt,
                    )

            def drain(ps, sblk, split):
                if split:
                    # tail path: drain j-halves independently so the
                    # first half's add+store overlaps the last matmul
                    for jh in range(2):
                        ot = opool.tile([P, 512], f32, name="oth", tag="oth")
                        nc.vector.tensor_add(
                            ot[:],
                            ps[:, jh * 512 : (jh + 1) * 512],
                            bias_t[:, jh * 512 : (jh + 1) * 512],
                        )
                        eng = nc.scalar if jh == 0 else nc.gpsimd
                        eng.dma_start(
                            out_d[
                                sblk * P : (sblk + 1) * P,
                                jh * 512 : (jh + 1) * 512,
                            ],
                            ot[:],
                        )
                else:
                    ot = opool.tile([P, DIM], f32, name="ot", tag="ot")
                    nc.vector.tensor_add(ot[:], ps[:], bias_t[:])
                    nc.scalar.dma_start(
                        out_d[sblk * P : (sblk + 1) * P, :], ot[:]
                    )

            bias_t = bpool.tile([P, DIM], f32)

            # PE warmup: 15 throwaway fp16 matmuls from ~7.5us. The
            # first ~9 run cold (427ns) and fill the HAM 3.4us busy
            # window (clock warms ~11.8us); the rest run warm and
            # bridge to first-data (~12.8us) so real matmuls start at
            # the 216ns warm rate with no cold era and no added delay.
            wt = bpool.tile([P, 512], f32r, name="wt")
            nc.gpsimd.memset(wt[:], 0.0)
            wps = psumpool.tile([P, 512], f32, tag="ps3", name="wps")
            for _ in range(15):
                nc.tensor.matmul(
                    wps[:], wt[:, :P], wt[:], start=True, stop=True
                )

            SLAB_SIZES = [512] * 7 + [256] * 2
            sblk0 = 0
            s_off = 0
            for slab, ssz in enumerate(SLAB_SIZES):
                n_m = ssz // P
                xts = []
                for kt in range(KT):
                    if slab == 0:
                        qeng = nc.gpsimd if kt % 2 == 0 else nc.scalar
                        for jh in range(2):
                            qt = qpool.tile(
                                [P, 512], f32r, tag=f"q{kt}_{jh}", name="qt"
                            )
                            qeng.dma_start(
                                qt[:],
                                q_r[:, kt, jh * 512 : (jh + 1) * 512],
                            )
                            q_tiles[kt][jh] = qt
                        if kt == 0:
                            nc.scalar.dma_start(bias_t[:], bias_d[:])
                    xt = xpool.tile([P, ssz], f32r, tag=f"x{kt}", name="xt")
                    nc.sync.dma_start(
                        xt[:],
                        xT_r[:, kt, s_off : s_off + ssz],
                    )
                    xts.append(xt)
                pss = [
                    psumpool.tile(
                        [P, DIM], f32, tag=f"ps{m % 4}", name=f"ps{m % 4}"
                    )
                    for m in range(n_m)
                ]
                last_slab = slab == len(SLAB_SIZES) - 1
                if slab == 0:
                    # kt-major: m-groups accumulate in parallel PSUM banks
                    for kt in range(KT):
                        for m in range(n_m):
                            mm(pss[m], xts, kt, m, kt == KT - 1)
                    for m in range(n_m):
                        drain(pss[m], sblk0 + m, False)
                else:
                    for m in range(n_m):
                        for kt in range(KT):
                            mm(pss[m], xts, kt, m, kt == KT - 1)
                        drain(
                            pss[m],
                            sblk0 + m,
                            last_slab and m == n_m - 1,
                        )
                sblk0 += n_m
                s_off += ssz

    nc.compile()
    return nc


def _get_nc():
    global _compiled_nc
    if _compiled_nc is None:
        _compiled_nc = _build_kernel()
    return _compiled_nc


def _cayley_q(angles: np.ndarray) -> np.ndarray:
    A = np.zeros((DIM, DIM), dtype=np.float64)
    iu = np.triu_indices(DIM, k=1)
    A[iu] = angles.astype(np.float64)
    A = A - A.T
    I = np.eye(DIM, dtype=np.float64)
    return np.linalg.solve(I - A, I + A).astype(np.float32)


def _run(inputs: dict, trace: bool = False, tmpdir: str | None = None):
    from concourse.bass_utils import run_bass_kernel_spmd

    x = np.asarray(inputs["x"], dtype=np.float32)
    angles = np.asarray(inputs["angles"], dtype=np.float32)
    bias = np.asarray(inputs["bias"], dtype=np.float32)

    Q = _cayley_q(angles).astype(np.float16)
    bias_rep = np.ascontiguousarray(
        np.broadcast_to(bias.astype(np.float32), (P, DIM))
    )
    in_maps = []
    for b in range(B):
        xT = np.ascontiguousarray(x[b].T.astype(np.float16))  # [1024, 4096]
        in_maps.append({"xT": xT, "q": Q, "biasr": bias_rep})

    nc = _get_nc()
    res = run_bass_kernel_spmd(
        nc, in_maps, list(range(N_CORES)), trace=trace, tmpdir=tmpdir
    )
    out = np.stack([res.results[b]["out"] for b in range(B)], axis=0)
    return out, res


def kernel(x, angles, bias):
    out, _ = _run({"x": x, "angles": angles, "bias": bias})
    return out



# revision 3
# speedup vs baseline: 1.3837x; 1.3837x over previous
"""CayleyLinear Trainium2 kernel — fp8-DoubleRow hybrid.

Computes out = x @ Q + bias where Q = (I-A)^-1 (I+A) is the Cayley
transform of the skew-symmetric matrix built from `angles`.

Strategy (data-parallel over batch, 8 NeuronCores):
  - Host: build A, solve for Q (tiny vs the 68.7 GFLOP matmul).
  - Identity+diagonal split: Q = D + R with D = diag(Q), R zero-diag.
    Device computes x @ R; the exact part x*D + bias rides in at fp16
    through a fused DVE op. This attenuates fp8 quantization noise by
    ||R||_F/sqrt(DIM) ~ 0.58.
  - Hybrid precision on the contraction: k-tiles 0..5 in fp8-e4m3 using
    DoubleRow perf mode (2 k-subtiles per matmul, 2x PE rate), k-tiles
    6..7 in fp16. Measured end-to-end rel err 1.82e-2 (gate 2e-2).
  - R is scaled by 512 before fp8/fp16 quantization so its entries
    (RMS ~0.018) land in e4m3's normal range; the drain multiplies by
    1/512 in the same fused op: out = (psum * 1/512) + (x*D + bias).
  - PE per core: 64 (m,jh) x (3 DoubleRow + 2 fp16) matmuls of 512
    moving rows = 5/8 of the pure-fp16 cycle count.
  - Output stored fp16 (rel err 2.4e-4, negligible), host casts to f32.
"""

import numpy as np

DIM = 1024
B = 8
S = 4096
N_CORES = 8
P = 128
KT8 = 6  # fp8 k-tiles (3 DoubleRow pairs)
KT16 = 2  # fp16 k-tiles
K8 = KT8 * P  # 768
RS = 512.0  # R pre-scale (power of 2; undone in the drain)

_compiled_nc = None


def _build_kernel():
    import concourse.bass as bass
    import concourse.mybir as mybir
    import concourse.tile as tile
    from concourse import bacc

    f32 = mybir.dt.float32
    f16 = mybir.dt.float16
    f8 = mybir.dt.float8e4
    DR = mybir.MatmulPerfMode.DoubleRow
    MULT = mybir.AluOpType.mult
    ADD = mybir.AluOpType.add

    nc = bacc.Bacc(
        "TRN2",
        target_bir_lowering=False,
        debug=False,
        num_devices=N_CORES,
        enable_partition_id=False,
    )

    x8_d = nc.dram_tensor("x8", [K8, S], f8, kind="ExternalInput").ap()
    x16_d = nc.dram_tensor("x16", [KT16 * P, S], f16, kind="ExternalInput").ap()
    r8_d = nc.dram_tensor("r8", [K8, DIM], f8, kind="ExternalInput").ap()
    r16_d = nc.dram_tensor("r16", [KT16 * P, DIM], f16, kind="ExternalInput").ap()
    xbd_d = nc.dram_tensor("xbd", [S, DIM], f16, kind="ExternalInput").ap()
    out_d = nc.dram_tensor("out", [S, DIM], f16, kind="ExternalOutput").ap()

    x8_r = x8_d.rearrange("(kt p) s -> p kt s", p=P)  # [128, 6, 4096]
    x16_r = x16_d.rearrange("(kt p) s -> p kt s", p=P)  # [128, 2, 4096]
    r8_r = r8_d.rearrange("(kt p) j -> p kt j", p=P)  # [128, 6, 1024]
    r16_r = r16_d.rearrange("(kt p) j -> p kt j", p=P)  # [128, 2, 1024]
    xbd_r = xbd_d.rearrange("(sb p) j -> p sb j", p=P)  # [128, 32, 1024]

    with tile.TileContext(nc) as tc:
        with (
            tc.tile_pool(name="rpool", bufs=1) as rpool,
            tc.tile_pool(name="xpool", bufs=3) as xpool,
            tc.tile_pool(name="xbpool", bufs=2) as xbpool,
            tc.tile_pool(name="opool", bufs=4) as opool,
            tc.tile_pool(name="psum", bufs=1, space="PSUM") as psumpool,
        ):
            r8_t = rpool.tile([P, KT8, DIM], f8)
            r16_t = rpool.tile([P, KT16, DIM], f16)

            def mm(ps, x8s, x16s, m, jh):
                # full K accumulation for one (m-block, j-half)
                jsl = slice(jh * 512, (jh + 1) * 512)
                msl = slice(m * P, (m + 1) * P)
                for kp in range(KT8 // 2):
                    ksl = slice(2 * kp, 2 * kp + 2)
                    nc.tensor.matmul(
                        ps[:, jsl],
                        x8s[:, ksl, msl],
                        r8_t[:, ksl, jsl],
                        start=(kp == 0),
                        stop=False,
                        perf_mode=DR,
                    )
                for kt in range(KT16):
                    nc.tensor.matmul(
                        ps[:, jsl],
                        x16s[:, kt, msl],
                        r16_t[:, kt, jsl],
                        start=False,
                        stop=(kt == KT16 - 1),
                    )

            def drain(ps, xbds, mi, sblk, split):
                # GPSIMD can't touch PSUM: j-half 0 goes scalar-ACT
                # (PSUM->SBUF fp16 with the 1/RS scale) + gpsimd fp16
                # add; j-half 1 is one fused DVE op. Spreads the drain
                # over three engines.
                Copy = mybir.ActivationFunctionType.Copy
                ot = opool.tile([P, DIM], f16, name="ot", tag="ot")
                t16 = opool.tile([P, 512], f16, name="t16", tag="t16")
                nc.scalar.activation(
                    t16[:], ps[:, :512], Copy, scale=1.0 / RS
                )
                nc.gpsimd.tensor_add(ot[:, :512], t16[:], xbds[:, mi, :512])
                nc.vector.scalar_tensor_tensor(
                    ot[:, 512:], ps[:, 512:], 1.0 / RS,
                    xbds[:, mi, 512:], MULT, ADD,
                )
                if split:
                    nc.scalar.dma_start(
                        out_d[sblk * P : (sblk + 1) * P, :512], ot[:, :512]
                    )
                    nc.gpsimd.dma_start(
                        out_d[sblk * P : (sblk + 1) * P, 512:], ot[:, 512:]
                    )
                else:
                    eng = nc.scalar if sblk % 2 == 0 else nc.gpsimd
                    eng.dma_start(out_d[sblk * P : (sblk + 1) * P, :], ot[:])

            # R loads first (resident all kernel): r8 on gpsimd, r16 on
            # scalar queue; x slabs stream on sync.
            nc.gpsimd.dma_start(r8_t[:], r8_r[:])
            nc.scalar.dma_start(r16_t[:], r16_r[:])

            # PE warmup: throwaway DoubleRow matmuls bridge the clock
            # ramp so real matmuls start at the warm rate.
            wt = rpool.tile([P, 2, 512], f8, name="wt")
            nc.gpsimd.memset(wt[:], 0.0)
            wps = psumpool.tile([P, 512], f32, tag="ps3", name="wps")
            for _ in range(15):
                nc.tensor.matmul(
                    wps[:], wt[:, :, :P], wt[:], start=True, stop=True,
                    perf_mode=DR,
                )

            SLAB_SIZES = [512] * 7 + [256] * 2
            sblk0 = 0
            s_off = 0
            for slab, ssz in enumerate(SLAB_SIZES):
                n_m = ssz // P
                x8s = xpool.tile([P, KT8, ssz], f8, tag="x8s", name="x8s")
                nc.sync.dma_start(x8s[:], x8_r[:, :, s_off : s_off + ssz])
                x16s = xpool.tile([P, KT16, ssz], f16, tag="x16s", name="x16s")
                nc.sync.dma_start(x16s[:], x16_r[:, :, s_off : s_off + ssz])
                xbds = xbpool.tile([P, n_m, DIM], f16, tag="xbd", name="xbds")
                nc.sync.dma_start(
                    xbds[:], xbd_r[:, sblk0 : sblk0 + n_m, :]
                )
                pss = [
                    psumpool.tile(
                        [P, DIM], f32, tag=f"ps{m % 4}", name=f"ps{m % 4}"
                    )
                    for m in range(n_m)
                ]
                last_slab = slab == len(SLAB_SIZES) - 1
                for m in range(n_m):
                    for jh in range(2):
                        mm(pss[m], x8s, x16s, m, jh)
                    drain(
                        pss[m],
                        xbds,
                        m,
                        sblk0 + m,
                        last_slab and m == n_m - 1,
                    )
                sblk0 += n_m
                s_off += ssz

    nc.compile()
    return nc


def _get_nc():
    global _compiled_nc
    if _compiled_nc is None:
        _compiled_nc = _build_kernel()
    return _compiled_nc


def _cayley_q(angles: np.ndarray) -> np.ndarray:
    A = np.zeros((DIM, DIM), dtype=np.float64)
    iu = np.triu_indices(DIM, k=1)
    A[iu] = angles.astype(np.float64)
    A = A - A.T
    I = np.eye(DIM, dtype=np.float64)
    return np.linalg.solve(I - A, I + A)


def _run(inputs: dict, trace: bool = False, tmpdir: str | None = None):
    import ml_dtypes
    from concourse.bass_utils import run_bass_kernel_spmd

    f8np = ml_dtypes.float8_e4m3

    x = np.asarray(inputs["x"], dtype=np.float32)
    angles = np.asarray(inputs["angles"], dtype=np.float32)
    bias = np.asarray(inputs["bias"], dtype=np.float32)

    Q = _cayley_q(angles)
    d = np.diag(Q).copy()
    R = Q - np.diag(d)  # zero diagonal
    Rs = (R * RS).astype(np.float32)
    r8 = np.ascontiguousarray(Rs[:K8]).astype(f8np)
    r16 = np.ascontiguousarray(Rs[K8:]).astype(np.float16)
    d32 = d.astype(np.float32)
    b32 = bias.astype(np.float32)

    in_maps = []
    for b in range(B):
        xT = np.ascontiguousarray(x[b].T)  # [1024, 4096] f32
        in_maps.append(
            {
                "x8": xT[:K8].astype(f8np),
                "x16": xT[K8:].astype(np.float16),
                "r8": r8,
                "r16": r16,
                "xbd": (x[b] * d32[None, :] + b32[None, :]).astype(
                    np.float16
                ),
            }
        )

    nc = _get_nc()
    res = run_bass_kernel_spmd(
        nc, in_maps, list(range(N_CORES)), trace=trace, tmpdir=tmpdir
    )
    out = np.stack(
        [res.results[b]["out"].astype(np.float32) for b in range(B)], axis=0
    )
    return out, res


def kernel(x, angles, bias):
    out, _ = _run({"x": x, "angles": angles, "bias": bias})
    return out


# revision 4
# speedup vs baseline: 1.4178x; 1.0247x over previous
"""CayleyLinear Trainium2 kernel — fp8-DoubleRow hybrid.

Computes out = x @ Q + bias where Q = (I-A)^-1 (I+A) is the Cayley
transform of the skew-symmetric matrix built from `angles`.

Strategy (data-parallel over batch, 8 NeuronCores):
  - Host: build A, solve for Q (tiny vs the 68.7 GFLOP matmul).
  - Identity+diagonal split: Q = D + R with D = diag(Q), R zero-diag.
    Device computes x @ R; the exact part x*D + bias rides in at fp16
    through a fused DVE op. This attenuates fp8 quantization noise by
    ||R||_F/sqrt(DIM) ~ 0.58.
  - Hybrid precision on the contraction: k-tiles 0..5 in fp8-e4m3 using
    DoubleRow perf mode (2 k-subtiles per matmul, 2x PE rate), k-tiles
    6..7 in fp16. Measured end-to-end rel err 1.82e-2 (gate 2e-2).
  - R is scaled by 512 before fp8/fp16 quantization so its entries
    (RMS ~0.018) land in e4m3's normal range; the drain multiplies by
    1/512 in the same fused op: out = (psum * 1/512) + (x*D + bias).
  - PE per core: 64 (m,jh) x (3 DoubleRow + 2 fp16) matmuls of 512
    moving rows = 5/8 of the pure-fp16 cycle count.
  - Output stored fp16 (rel err 2.4e-4, negligible), host casts to f32.
"""

import numpy as np

DIM = 1024
B = 8
S = 4096
N_CORES = 8
P = 128
KT8 = 6  # fp8 k-tiles (3 DoubleRow pairs)
KT16 = 2  # fp16 k-tiles
K8 = KT8 * P  # 768
RS = 512.0  # R pre-scale (power of 2; undone in the drain)

_compiled_nc = None


def _build_kernel():
    import concourse.bass as bass
    import concourse.mybir as mybir
    import concourse.tile as tile
    from concourse import bacc

    f32 = mybir.dt.float32
    f16 = mybir.dt.float16
    f8 = mybir.dt.float8e4
    DR = mybir.MatmulPerfMode.DoubleRow
    MULT = mybir.AluOpType.mult
    ADD = mybir.AluOpType.add

    nc = bacc.Bacc(
        "TRN2",
        target_bir_lowering=False,
        debug=False,
        num_devices=N_CORES,
        enable_partition_id=False,
    )

    x8_d = nc.dram_tensor("x8", [K8, S], f8, kind="ExternalInput").ap()
    x16_d = nc.dram_tensor("x16", [KT16 * P, S], f16, kind="ExternalInput").ap()
    r8_d = nc.dram_tensor("r8", [K8, DIM], f8, kind="ExternalInput").ap()
    r16_d = nc.dram_tensor("r16", [KT16 * P, DIM], f16, kind="ExternalInput").ap()
    xbd_d = nc.dram_tensor("xbd", [S, DIM], f16, kind="ExternalInput").ap()
    out_d = nc.dram_tensor("out", [S, DIM], f16, kind="ExternalOutput").ap()

    x8_r = x8_d.rearrange("(kt p) s -> p kt s", p=P)  # [128, 6, 4096]
    x16_r = x16_d.rearrange("(kt p) s -> p kt s", p=P)  # [128, 2, 4096]
    r8_r = r8_d.rearrange("(kt p) j -> p kt j", p=P)  # [128, 6, 1024]
    r16_r = r16_d.rearrange("(kt p) j -> p kt j", p=P)  # [128, 2, 1024]
    xbd_r = xbd_d.rearrange("(sb p) j -> p sb j", p=P)  # [128, 32, 1024]

    with tile.TileContext(nc) as tc:
        with (
            tc.tile_pool(name="rpool", bufs=1) as rpool,
            tc.tile_pool(name="xpool", bufs=3) as xpool,
            tc.tile_pool(name="xbpool", bufs=2) as xbpool,
            tc.tile_pool(name="opool", bufs=4) as opool,
            tc.tile_pool(name="psum", bufs=1, space="PSUM") as psumpool,
        ):
            r8_t = rpool.tile([P, KT8, DIM], f8)
            r16_t = rpool.tile([P, KT16, DIM], f16)
            Copy = mybir.ActivationFunctionType.Copy

            def mm(ps, x8s, x16s, m, jh):
                # full K accumulation for one (m-block, j-half) into a
                # single [128,512] PSUM bank
                jsl = slice(jh * 512, (jh + 1) * 512)
                msl = slice(m * P, (m + 1) * P)
                for kp in range(KT8 // 2):
                    ksl = slice(2 * kp, 2 * kp + 2)
                    nc.tensor.matmul(
                        ps[:],
                        x8s[:, ksl, msl],
                        r8_t[:, ksl, jsl],
                        start=(kp == 0),
                        stop=False,
                        perf_mode=DR,
                    )
                for kt in range(KT16):
                    nc.tensor.matmul(
                        ps[:],
                        x16s[:, kt, msl],
                        r16_t[:, kt, jsl],
                        start=False,
                        stop=(kt == KT16 - 1),
                    )

            def drain(ps, xbds, mi, sblk, jh):
                # j-half 0: scalar-ACT scales PSUM->SBUF fp16 (frees the
                # bank fast), DVE does the fp16 add.  j-half 1: one
                # fused DVE op.  Stores split sync/gpsimd.
                jsl = slice(jh * 512, (jh + 1) * 512)
                ot = opool.tile([P, 512], f16, name="ot", tag=f"ot{jh}")
                if jh == 0:
                    t16 = opool.tile([P, 512], f16, name="t16", tag="t16")
                    nc.scalar.activation(t16[:], ps[:], Copy, scale=1.0 / RS)
                    nc.vector.tensor_add(ot[:], t16[:], xbds[:, mi, jsl])
                    nc.sync.dma_start(
                        out_d[sblk * P : (sblk + 1) * P, jsl], ot[:]
                    )
                else:
                    nc.vector.scalar_tensor_tensor(
                        ot[:], ps[:], 1.0 / RS, xbds[:, mi, jsl], MULT, ADD
                    )
                    nc.gpsimd.dma_start(
                        out_d[sblk * P : (sblk + 1) * P, jsl], ot[:]
                    )

            # Warmup weights memset is the first gpsimd instruction so
            # PE warmups start right after the preamble; R tiles load
            # next (r8 split over the gpsimd+scalar queues); x slabs
            # stream on sync, xbd slabs on gpsimd.
            wt = rpool.tile([P, 2, 512], f8, name="wt")
            nc.gpsimd.memset(wt[:], 0.0)
            nc.gpsimd.dma_start(r8_t[:, :3, :], r8_r[:, :3, :])
            nc.scalar.dma_start(r8_t[:, 3:, :], r8_r[:, 3:, :])
            nc.scalar.dma_start(r16_t[:], r16_r[:])

            # PE warmup: throwaway DoubleRow matmuls bridge the clock
            # ramp so real matmuls start at the warm rate.
            wps = psumpool.tile([P, 512], f32, tag="ps31", name="wps")
            for _ in range(12):
                nc.tensor.matmul(
                    wps[:], wt[:, :, :P], wt[:], start=True, stop=True,
                    perf_mode=DR,
                )

            SLAB_SIZES = [512] * 7 + [256, 128, 128]
            sblk0 = 0
            s_off = 0
            for slab, ssz in enumerate(SLAB_SIZES):
                n_m = ssz // P
                x8s = xpool.tile([P, KT8, ssz], f8, tag="x8s", name="x8s")
                nc.sync.dma_start(x8s[:], x8_r[:, :, s_off : s_off + ssz])
                x16s = xpool.tile([P, KT16, ssz], f16, tag="x16s", name="x16s")
                nc.sync.dma_start(x16s[:], x16_r[:, :, s_off : s_off + ssz])
                xbds = xbpool.tile([P, n_m, DIM], f16, tag="xbd", name="xbds")
                nc.gpsimd.dma_start(
                    xbds[:], xbd_r[:, sblk0 : sblk0 + n_m, :]
                )
                pss = [
                    [
                        psumpool.tile(
                            [P, 512], f32,
                            tag=f"ps{m % 4}{jh}", name=f"ps{m % 4}{jh}",
                        )
                        for jh in range(2)
                    ]
                    for m in range(n_m)
                ]
                for m in range(n_m):
                    for jh in range(2):
                        mm(pss[m][jh], x8s, x16s, m, jh)
                        drain(pss[m][jh], xbds, m, sblk0 + m, jh)
                sblk0 += n_m
                s_off += ssz

    nc.compile()
    return nc


def _get_nc():
    global _compiled_nc
    if _compiled_nc is None:
        _compiled_nc = _build_kernel()
    return _compiled_nc


def _cayley_q(angles: np.ndarray) -> np.ndarray:
    A = np.zeros((DIM, DIM), dtype=np.float64)
    iu = np.triu_indices(DIM, k=1)
    A[iu] = angles.astype(np.float64)
    A = A - A.T
    I = np.eye(DIM, dtype=np.float64)
    return np.linalg.solve(I - A, I + A)


def _run(inputs: dict, trace: bool = False, tmpdir: str | None = None):
    import ml_dtypes
    from concourse.bass_utils import run_bass_kernel_spmd

    f8np = ml_dtypes.float8_e4m3

    x = np.asarray(inputs["x"], dtype=np.float32)
    angles = np.asarray(inputs["angles"], dtype=np.float32)
    bias = np.asarray(inputs["bias"], dtype=np.float32)

    Q = _cayley_q(angles)
    d = np.diag(Q).copy()
    R = Q - np.diag(d)  # zero diagonal
    Rs = (R * RS).astype(np.float32)
    r8 = np.ascontiguousarray(Rs[:K8]).astype(f8np)
    r16 = np.ascontiguousarray(Rs[K8:]).astype(np.float16)
    d32 = d.astype(np.float32)
    b32 = bias.astype(np.float32)

    in_maps = []
    for b in range(B):
        xT = np.ascontiguousarray(x[b].T)  # [1024, 4096] f32
        in_maps.append(
            {
                "x8": xT[:K8].astype(f8np),
                "x16": xT[K8:].astype(np.float16),
                "r8": r8,
                "r16": r16,
                "xbd": (x[b] * d32[None, :] + b32[None, :]).astype(
                    np.float16
                ),
            }
        )

    nc = _get_nc()
    res = run_bass_kernel_spmd(
        nc, in_maps, list(range(N_CORES)), trace=trace, tmpdir=tmpdir
    )
    out = np.stack(
        [res.results[b]["out"].astype(np.float32) for b in range(B)], axis=0
    )
    return out, res


def kernel(x, angles, bias):
    out, _ = _run({"x": x, "angles": angles, "bias": bias})
    return out
